# revision 1
# baseline (speedup 1.0000x reference)
"""minGRU Trainium2 Bass kernel.

Reference computation (per batch b):
    hidden = x @ W_hidden            [S, Di]
    gate   = x @ W_gate              [S, Di]
    a_t    = sigmoid(-gate)          (= exp(log_coeffs) = 1 - z)
    z_t    = sigmoid(gate)
    g(h)   = h + 0.5 if h >= 0 else sigmoid(h)
    b_t    = z_t * g(hidden_t)
    h_t    = a_t * h_{t-1} + b_t     (h_{-1} = 0; linear-space scan,
                                      numerically stable: convex combination)
    out    = h @ W_out               [S, D]

Sharding over 8 cores: (batch b in 0..3) x (half of Di). Each core computes
its batch's projections against its 768-column slice of W_hidden/W_gate,
scans, and multiplies by its 768-row slice of W_out, producing a partial
[D, S] (transposed) output. Host adds the two halves and transposes back.

Device layout: everything keeps the sequence on the free axis and
features/d-model on partitions, so no on-device transposes are needed:
    x is fed pre-transposed as xT [D, S];
    proj matmul: out[f, s] = sum_d Wh[d, f] * xT[d, s]  (lhsT = Wh, rhs = xT)
    scan: tensor_tensor_scan along the free (sequence) axis
    out matmul: outT[d, s] = sum_f Wo[f, d] * h[f, s]   (lhsT = Wo, rhs = h)
Matmuls run in float32r (full fp32 data, reduced-precision PE multiply,
1 cycle/row vs 4 for strict fp32).
"""

import numpy as np
from contextlib import ExitStack

import concourse.bass as bass
import concourse.tile as tile
from concourse import bacc, mybir
from concourse.bass_utils import run_bass_kernel_spmd

B = 4
S = 4096
D = 1024
DI = 1536
F = DI // 2            # 768 features per core
N_CORES = 8
SC = 512               # sequence chunk (one PSUM bank of fp32)
KD = D // 128          # 8 contraction tiles for the projections
NF = F // 128          # 6 feature tiles per core
ND = D // 128          # 8 output d-model tiles

F32 = mybir.dt.float32
F32R = mybir.dt.float32r
ACT = mybir.ActivationFunctionType
ALU = mybir.AluOpType

_cache = {}


def _build(seq_len=S, matmul_dtype=F32R, reps=1, timing=False):
    nsc = seq_len // SC
    nc = bacc.Bacc("TRN2", target_bir_lowering=False, debug=False,
                   num_devices=N_CORES)
    md = matmul_dtype
    if timing:
        # Timing build: all big tensors stay device-internal (their values
        # are irrelevant for speed) so repeated calls ship only a token
        # through the axon tunnel.
        xT = nc.dram_tensor("xT", [D, seq_len], md).ap()
        wh = nc.dram_tensor("wh", [D, F], md).ap()
        wg = nc.dram_tensor("wg", [D, F], md).ap()
        wo = nc.dram_tensor("wo", [F, D], md).ap()
        outT = nc.dram_tensor("outT", [D, seq_len], F32).ap()
        seed = nc.dram_tensor("seed", [1, 8], F32, kind="ExternalInput").ap()
        done = nc.dram_tensor("done", [1, 8 * reps], F32,
                              kind="ExternalOutput").ap()
    else:
        xT = nc.dram_tensor("xT", [D, seq_len], md, kind="ExternalInput").ap()
        wh = nc.dram_tensor("wh", [D, F], md, kind="ExternalInput").ap()
        wg = nc.dram_tensor("wg", [D, F], md, kind="ExternalInput").ap()
        wo = nc.dram_tensor("wo", [F, D], md, kind="ExternalInput").ap()
        outT = nc.dram_tensor("outT", [D, seq_len], F32,
                              kind="ExternalOutput").ap()
        seed = None
        done = None

    with tile.TileContext(nc) as tc, ExitStack() as ctx:
        wpool = ctx.enter_context(tc.tile_pool(name="w", bufs=1))
        xpool = ctx.enter_context(tc.tile_pool(name="x", bufs=3))
        ppool = ctx.enter_context(tc.tile_pool(name="pp", bufs=2, space="PSUM"))
        opool = ctx.enter_context(tc.tile_pool(name="po", bufs=4, space="PSUM"))
        epool = ctx.enter_context(tc.tile_pool(name="e", bufs=3))
        hpool = ctx.enter_context(tc.tile_pool(name="h", bufs=2))
        spool = ctx.enter_context(tc.tile_pool(name="os", bufs=3))

        # Resident weights. Column block dk of wh_sb/wg_sb holds rows
        # dk*128..+128 of the [D, F] weight; block fk of wo_sb holds rows
        # fk*128..+128 of the [F, D] weight.
        wh_sb = wpool.tile([128, KD * F], md, tag="wh")
        wg_sb = wpool.tile([128, KD * F], md, tag="wg")
        wo_sb = wpool.tile([128, NF * D], md, tag="wo")
        for dk in range(KD):
            nc.sync.dma_start(wh_sb[:, dk * F:(dk + 1) * F],
                              wh[dk * 128:(dk + 1) * 128, :])
            nc.sync.dma_start(wg_sb[:, dk * F:(dk + 1) * F],
                              wg[dk * 128:(dk + 1) * 128, :])
        for fk in range(NF):
            nc.sync.dma_start(wo_sb[:, fk * D:(fk + 1) * D],
                              wo[fk * 128:(fk + 1) * 128, :])

        for _rep in range(reps):
          h_prev = [None] * NF
          for sc in range(nsc):
            x_sb = xpool.tile([128, KD * SC], md, tag="x")
            for dk in range(KD):
                nc.sync.dma_start(
                    x_sb[:, dk * SC:(dk + 1) * SC],
                    xT[dk * 128:(dk + 1) * 128, sc * SC:(sc + 1) * SC])

            h_cur = []
            for ft in range(NF):
                ph = ppool.tile([128, SC], F32, tag="ph")
                pg = ppool.tile([128, SC], F32, tag="pg")
                for dk in range(KD):
                    cw = dk * F + ft * 128
                    rx = x_sb[:, dk * SC:(dk + 1) * SC]
                    nc.tensor.matmul(
                        ph[:], wh_sb[:, cw:cw + 128],
                        rx, start=(dk == 0), stop=(dk == KD - 1))
                    nc.tensor.matmul(
                        pg[:], wg_sb[:, cw:cw + 128],
                        rx, start=(dk == 0), stop=(dk == KD - 1))

                z_sb = epool.tile([128, SC], F32, tag="z")
                a_sb = epool.tile([128, SC], F32, tag="a")
                s_sb = epool.tile([128, SC], F32, tag="s")
                r_sb = epool.tile([128, SC], F32, tag="r")
                g_sb = epool.tile([128, SC], F32, tag="g")
                b_sb = epool.tile([128, SC], F32, tag="b")
                nc.scalar.activation(z_sb[:], pg[:], ACT.Sigmoid)
                nc.scalar.activation(a_sb[:], pg[:], ACT.Sigmoid, scale=-1.0)
                nc.scalar.activation(s_sb[:], ph[:], ACT.Sigmoid)
                nc.scalar.activation(r_sb[:], ph[:], ACT.Relu)
                # g = min(sigmoid(h), 0.5) + relu(h)
                nc.vector.scalar_tensor_tensor(
                    g_sb[:], s_sb[:], 0.5, r_sb[:], op0=ALU.min, op1=ALU.add)
                nc.vector.tensor_mul(b_sb[:], z_sb[:], g_sb[:])

                h_sb = hpool.tile([128, SC], md, tag=f"h{ft}")
                init = 0.0 if sc == 0 else h_prev[ft][:, SC - 1:SC]
                nc.vector.tensor_tensor_scan(
                    h_sb[:], a_sb[:], b_sb[:], init,
                    op0=ALU.mult, op1=ALU.add)
                h_cur.append(h_sb)

            for dt_ in range(ND):
                po = opool.tile([128, SC], F32, tag="po")
                for fk in range(NF):
                    cw = fk * D + dt_ * 128
                    nc.tensor.matmul(
                        po[:], wo_sb[:, cw:cw + 128],
                        h_cur[fk][:],
                        start=(fk == 0), stop=(fk == NF - 1))
                o_sb = spool.tile([128, SC], F32, tag="o")
                # PSUM has no DMA route; alternate the copy between engines.
                if dt_ % 2 == 0:
                    nc.scalar.copy(o_sb[:], po[:])
                else:
                    nc.vector.tensor_copy(o_sb[:], po[:])
                nc.sync.dma_start(
                    outT[dt_ * 128:(dt_ + 1) * 128, sc * SC:(sc + 1) * SC],
                    o_sb[:])
            h_prev = h_cur

          if timing and _rep == reps - 1:
            tok = spool.tile([1, 8 * reps], F32, tag="tok")
            nc.vector.memset(tok[:], 1.0)
            nc.sync.dma_start(done[:], tok[:])

    nc.compile()
    return nc


def get_nc(seq_len=S, matmul_dtype=F32R, reps=1, timing=False):
    key = (seq_len, matmul_dtype, reps, timing)
    if key not in _cache:
        _cache[key] = _build(seq_len, matmul_dtype, reps, timing)
    return _cache[key]


def make_in_maps(x, W_hidden, W_gate, W_out, matmul_dtype=F32R):
    """Shard full inputs into per-core input maps (core c -> batch c//2,
    Di-half c%2)."""
    np_dt = mybir.dt.np(matmul_dtype)
    in_maps = []
    xT = np.ascontiguousarray(np.transpose(x, (0, 2, 1))).astype(np_dt)
    for c in range(N_CORES):
        b, hf = divmod(c, 2)
        in_maps.append({
            "xT": xT[b],
            "wh": np.ascontiguousarray(W_hidden[:, hf * F:(hf + 1) * F]).astype(np_dt),
            "wg": np.ascontiguousarray(W_gate[:, hf * F:(hf + 1) * F]).astype(np_dt),
            "wo": np.ascontiguousarray(W_out[hf * F:(hf + 1) * F, :]).astype(np_dt),
        })
    return in_maps


def assemble(results):
    """Combine per-core partial transposed outputs into [B, S, D]."""
    out = np.empty((B, S, D), np.float32)
    for b in range(B):
        acc = results[2 * b]["outT"] + results[2 * b + 1]["outT"]  # [D, S]
        out[b] = acc.T
    return out


def kernel(x, W_hidden, W_gate, W_out):
    x = np.asarray(x, np.float32)
    W_hidden = np.asarray(W_hidden, np.float32)
    W_gate = np.asarray(W_gate, np.float32)
    W_out = np.asarray(W_out, np.float32)
    nc = get_nc()
    in_maps = make_in_maps(x, W_hidden, W_gate, W_out)
    last_err = None
    for attempt in range(3):
        try:
            res = run_bass_kernel_spmd(nc, in_maps, list(range(N_CORES)))
            return assemble(res.results)
        except Exception as e:  # transient device faults under axon
            last_err = e
            import time as _time
            _time.sleep(5.0 * (attempt + 1))
    raise last_err



# revision 8
# speedup vs baseline: 1.4414x; 1.4414x over previous
"""minGRU Trainium2 Bass kernel (mixed fp8/bf16).

Reference computation (per batch b):
    hidden = x @ W_hidden            [S, Di]
    gate   = x @ W_gate              [S, Di]
    a_t    = sigmoid(-gate)          (= 1 - z)
    z_t    = sigmoid(gate)
    g(h)   = h + 0.5 if h >= 0 else sigmoid(h)
    b_t    = z_t * g(hidden_t)
    h_t    = a_t * h_{t-1} + b_t     (h_{-1} = 0; linear-space scan)
    out    = h @ W_out               [S, D]

Sharding over 8 cores: (batch b in 0..3) x (half of Di). Each core computes
its batch's projections against its 768-column slice of W_hidden/W_gate,
scans, and multiplies by its 768-row slice of W_out, producing a partial
[D, S] (transposed) output. Host adds the two halves and transposes back.

Precision strategy (error budget: scale-rel max err < 2e-2):
  - gate proj: fp8e4m3 (x8, 32*Wg) with DoubleRow perf mode (2 k-tiles per
    matmul). Gate errors are damped by sigmoid'/scan -> tiny contribution.
  - hidden proj: bf16 (errors pass straight through g() into the scan).
  - out proj: per-core features split: first O8 128-tiles use fp8 DoubleRow
    on mean-centered h (dh8 = q8(16*(h - c)), c = per-feature mean of h over
    chunk 0, computed on device; correction c@W_out folded into the
    PSUM->SBUF copy as a per-partition bias), remaining tiles bf16 on h.
    Centering halves |dh| vs |h| -> halves the fp8 quantization error.
  Scales: fp8 weights x32, dh x16, bf16 W_out x512 so every out-proj path
  accumulates 512*out in PSUM; the copy applies (po + 512*c@Wo) * (1/512).
"""

import numpy as np
import ml_dtypes
from contextlib import ExitStack

import concourse.bass as bass
import concourse.tile as tile
from concourse import bacc, mybir
from concourse.bass_utils import run_bass_kernel_spmd

B = 4
S = 4096
D = 1024
DI = 1536
F = DI // 2            # 768 features per core
N_CORES = 8
SC = 512               # sequence chunk (one PSUM bank of fp32)
KD = D // 128          # 8 contraction tiles for the projections
NF = F // 128          # 6 feature tiles per core
ND = D // 128          # 8 output d-model tiles

G8 = 8                 # gate proj k-tiles in fp8 (even, 0..8)
H8 = 0                 # hidden proj k-tiles in fp8 (even, 0..8)
O8 = 4                 # out proj feature tiles in fp8 (even, 0..6)

SW = 32.0              # fp8 weight scale
SH = 16.0              # dh scale
OS = 512.0             # out-proj PSUM scale (= SW * SH)

F32 = mybir.dt.float32
BF16 = mybir.dt.bfloat16
FP8 = mybir.dt.float8e4
ACT = mybir.ActivationFunctionType
ALU = mybir.AluOpType
DR = mybir.MatmulPerfMode.DoubleRow

NP_FP8 = ml_dtypes.float8_e4m3
NP_BF16 = ml_dtypes.bfloat16

_cache = {}

def _emit_out(nc, d8_sb, hb_cur, sc, wo8_sb, wob_sb, v_sb, vd_sb,
              opool, spool, outT):
    """Out-projection + PSUM->SBUF copy + store for one chunk."""
    for dt_ in range(ND):
        po = opool.tile([128, SC], F32, tag="po")
        nmm = O8 // 2 + (NF - O8)
        i = 0
        for fk in range(0, O8, 2):
            nc.tensor.matmul(
                po[:], wo8_sb[:, fk:fk + 2, dt_ * 128:(dt_ + 1) * 128],
                d8_sb[:, fk:fk + 2, :], perf_mode=DR,
                start=(i == 0), stop=(i == nmm - 1))
            i += 1
        for fk in range(O8, NF):
            nc.tensor.matmul(
                po[:], wob_sb[:, fk, dt_ * 128:(dt_ + 1) * 128],
                hb_cur[fk][:],
                start=(i == 0), stop=(i == nmm - 1))
            i += 1
        o_sb = spool.tile([128, SC], F32, tag="o")
        # out = (po + 512*c@Wo) / 512, on Act (DVE is the busier engine)
        if O8:
            nc.scalar.activation(
                o_sb[:], po[:], ACT.Identity,
                bias=vd_sb[:, dt_:dt_ + 1], scale=1.0 / OS)
        else:
            nc.scalar.activation(o_sb[:], po[:], ACT.Copy, scale=1.0 / OS)
        nc.sync.dma_start(
            outT[dt_ * 128:(dt_ + 1) * 128, sc * SC:(sc + 1) * SC],
            o_sb[:])




def _build(seq_len=S, reps=1, timing=False):
    nsc = seq_len // SC
    nc = bacc.Bacc("TRN2", target_bir_lowering=False, debug=False,
                   num_devices=N_CORES)
    kind_in = None if timing else "ExternalInput"
    kind_out = None if timing else "ExternalOutput"

    def dram(name, shape, dt, kind):
        if kind is None:
            return nc.dram_tensor(name, shape, dt).ap()
        return nc.dram_tensor(name, shape, dt, kind=kind).ap()

    x8 = dram("x8", [D, seq_len], FP8, kind_in) if (G8 or H8) else None
    xb = dram("xb", [D, seq_len], BF16, kind_in) if (G8 < KD or H8 < KD) else None
    wg8 = dram("wg8", [G8 * 128, F], FP8, kind_in) if G8 else None
    wgb = dram("wgb", [(KD - G8) * 128, F], BF16, kind_in) if G8 < KD else None
    wh8 = dram("wh8", [H8 * 128, F], FP8, kind_in) if H8 else None
    whb = dram("whb", [(KD - H8) * 128, F], BF16, kind_in) if H8 < KD else None
    wo8 = dram("wo8", [O8 * 128, D], FP8, kind_in) if O8 else None
    wob = dram("wob", [F, D], BF16, kind_in)
    outT = dram("outT", [D, seq_len], F32, kind_out)
    if timing:
        seed = nc.dram_tensor("seed", [1, 8], F32, kind="ExternalInput").ap()
        done = nc.dram_tensor("done", [1, 8 * reps], F32,
                              kind="ExternalOutput").ap()

    with tile.TileContext(nc) as tc, ExitStack() as ctx:
        wpool = ctx.enter_context(tc.tile_pool(name="w", bufs=1))
        xpool = ctx.enter_context(tc.tile_pool(name="x", bufs=3))
        ppool = ctx.enter_context(tc.tile_pool(name="pp", bufs=2, space="PSUM"))
        opool = ctx.enter_context(tc.tile_pool(name="po", bufs=3, space="PSUM"))
        vpool = ctx.enter_context(tc.tile_pool(name="pv", bufs=1, space="PSUM"))
        epool = ctx.enter_context(tc.tile_pool(name="e", bufs=3))
        hpool = ctx.enter_context(tc.tile_pool(name="h", bufs=2))
        dpool = ctx.enter_context(tc.tile_pool(name="d8", bufs=2))
        spool = ctx.enter_context(tc.tile_pool(name="os", bufs=3))
        cpool = ctx.enter_context(tc.tile_pool(name="c", bufs=1))

        # Resident weights, 3D tiles: dim1 indexes the 128-row k-tile.
        if G8:
            wg8_sb = wpool.tile([128, G8, F], FP8, tag="wg8")
            for dk in range(G8):
                nc.sync.dma_start(wg8_sb[:, dk, :], wg8[dk * 128:(dk + 1) * 128, :])
        if G8 < KD:
            wgb_sb = wpool.tile([128, KD - G8, F], BF16, tag="wgb")
            for dk in range(KD - G8):
                nc.sync.dma_start(wgb_sb[:, dk, :], wgb[dk * 128:(dk + 1) * 128, :])
        if H8:
            wh8_sb = wpool.tile([128, H8, F], FP8, tag="wh8")
            for dk in range(H8):
                nc.sync.dma_start(wh8_sb[:, dk, :], wh8[dk * 128:(dk + 1) * 128, :])
        if H8 < KD:
            whb_sb = wpool.tile([128, KD - H8, F], BF16, tag="whb")
            for dk in range(KD - H8):
                nc.sync.dma_start(whb_sb[:, dk, :], whb[dk * 128:(dk + 1) * 128, :])
        if O8:
            wo8_sb = wpool.tile([128, O8, D], FP8, tag="wo8")
            for fk in range(O8):
                nc.sync.dma_start(wo8_sb[:, fk, :], wo8[fk * 128:(fk + 1) * 128, :])
        wob_sb = wpool.tile([128, NF, D], BF16, tag="wob")
        for fk in range(NF):
            nc.sync.dma_start(wob_sb[:, fk, :], wob[fk * 128:(fk + 1) * 128, :])

        # Per-partition bias tiles for the centered out-proj (chunk-0 c).
        if O8:
            negc_sb = cpool.tile([128, O8], F32, tag="negc")    # -16*c per F8 tile
            v_sb = cpool.tile([128, ND], F32, tag="v")          # 512*c@Wo
            vd_sb = cpool.tile([128, ND], F32, tag="vd")        # c@Wo

        for _rep in range(reps):
          h_prev = [None] * NF
          prev = None      # (d8_sb, hb_cur, sc) awaiting out-proj
          for sc in range(nsc):
            if G8 or H8:
                x8_sb = xpool.tile([128, KD, SC], FP8, tag="x8")
                for dk in range(KD):
                    nc.sync.dma_start(
                        x8_sb[:, dk, :],
                        x8[dk * 128:(dk + 1) * 128, sc * SC:(sc + 1) * SC])
            if G8 < KD or H8 < KD:
                xb_sb = xpool.tile([128, KD, SC], BF16, tag="xb")
                for dk in range(KD):
                    nc.sync.dma_start(
                        xb_sb[:, dk, :],
                        xb[dk * 128:(dk + 1) * 128, sc * SC:(sc + 1) * SC])

            h_cur = []
            hb_cur = {}
            if O8:
                d8_sb = dpool.tile([128, O8, SC], FP8, tag="d8")
            for ft in range(NF):
                ph = ppool.tile([128, SC], F32, tag="ph")
                pg = ppool.tile([128, SC], F32, tag="pg")
                cw = ft * 128
                # gate: fp8 DoubleRow pairs then bf16 remainder
                nmm = G8 // 2 + (KD - G8)
                i = 0
                for dk in range(0, G8, 2):
                    nc.tensor.matmul(
                        pg[:], wg8_sb[:, dk:dk + 2, cw:cw + 128],
                        x8_sb[:, dk:dk + 2, :], perf_mode=DR,
                        start=(i == 0), stop=(i == nmm - 1))
                    i += 1
                for dk in range(G8, KD):
                    nc.tensor.matmul(
                        pg[:], wgb_sb[:, dk - G8, cw:cw + 128],
                        xb_sb[:, dk, :],
                        start=(i == 0), stop=(i == nmm - 1))
                    i += 1
                # hidden: same split
                nmm = H8 // 2 + (KD - H8)
                i = 0
                for dk in range(0, H8, 2):
                    nc.tensor.matmul(
                        ph[:], wh8_sb[:, dk:dk + 2, cw:cw + 128],
                        x8_sb[:, dk:dk + 2, :], perf_mode=DR,
                        start=(i == 0), stop=(i == nmm - 1))
                    i += 1
                for dk in range(H8, KD):
                    nc.tensor.matmul(
                        ph[:], whb_sb[:, dk - H8, cw:cw + 128],
                        xb_sb[:, dk, :],
                        start=(i == 0), stop=(i == nmm - 1))
                    i += 1

                z_sb = epool.tile([128, SC], F32, tag="z")
                a_sb = epool.tile([128, SC], F32, tag="a")
                s_sb = epool.tile([128, SC], F32, tag="s")
                g_sb = epool.tile([128, SC], F32, tag="g")
                b_sb = epool.tile([128, SC], F32, tag="b")
                gs = 1.0 / SW if G8 else 1.0
                hs = 1.0 / SW if H8 else 1.0
                # PSUM readers must be Act or DVE (Pool is a slow software
                # engine with no PSUM access; unused here).
                nc.scalar.activation(z_sb[:], pg[:], ACT.Sigmoid, scale=gs)
                nc.scalar.activation(s_sb[:], ph[:], ACT.Sigmoid, scale=hs)
                # a = 1 - z (SBUF-only DVE op -> 2x mode)
                nc.vector.tensor_scalar(a_sb[:], z_sb[:], -1.0, 1.0,
                                        ALU.mult, ALU.add)
                # g = min(sigmoid(h), 0.5) + relu(h) == max(sigmoid(h), h+0.5)
                if H8:
                    nc.vector.tensor_scalar(g_sb[:], ph[:], hs, 0.5,
                                            ALU.mult, ALU.add)
                    nc.vector.tensor_tensor(g_sb[:], g_sb[:], s_sb[:],
                                            op=ALU.max)
                else:
                    nc.vector.scalar_tensor_tensor(
                        g_sb[:], ph[:], 0.5, s_sb[:],
                        op0=ALU.add, op1=ALU.max)
                nc.vector.tensor_mul(b_sb[:], z_sb[:], g_sb[:])

                h_sb = hpool.tile([128, SC], F32, tag=f"h{ft}")
                init = 0.0 if sc == 0 else h_prev[ft][:, SC - 1:SC]
                nc.vector.tensor_tensor_scan(
                    h_sb[:], a_sb[:], b_sb[:], init,
                    op0=ALU.mult, op1=ALU.add)
                h_cur.append(h_sb)

            if O8 and sc == 0:
                # c = per-feature mean of h over chunk 0 (bias matmuls are
                # deferred to the next chunk to keep the PE stream busy).
                cb_sb = cpool.tile([128, O8], BF16, tag="cb")
                for fk in range(O8):
                    hsum = epool.tile([128, 1], F32, tag="hsum")
                    nc.vector.tensor_reduce(
                        hsum[:], h_cur[fk][:], mybir.AxisListType.X, ALU.add)
                    nc.vector.tensor_scalar(negc_sb[:, fk:fk + 1], hsum[:],
                                            -1.0 / SC, None, ALU.mult)
                    nc.vector.tensor_scalar(cb_sb[:, fk:fk + 1], hsum[:],
                                            1.0 / SC, None, ALU.mult)

            for ft in range(NF):
                if ft < O8:
                    # dh8 = fp8(16*(h - c)) (SBUF-only DVE op)
                    nc.vector.tensor_scalar(
                        d8_sb[:, ft, :], h_cur[ft][:],
                        negc_sb[:, ft:ft + 1], SH, ALU.add, ALU.mult)
                else:
                    hb_sb = epool.tile([128, SC], BF16, tag=f"hb{ft}")
                    nc.vector.tensor_copy(hb_sb[:], h_cur[ft][:])
                    hb_cur[ft] = hb_sb

            if O8 and sc == min(1, nsc - 1):
                pv = vpool.tile([128, ND], F32, tag="pv")
                for dt_ in range(ND):
                    for fk in range(O8):
                        nc.tensor.matmul(
                            pv[:, dt_:dt_ + 1],
                            wob_sb[:, fk, dt_ * 128:(dt_ + 1) * 128],
                            cb_sb[:, fk:fk + 1],
                            start=(fk == 0), stop=(fk == O8 - 1))
                nc.vector.tensor_copy(v_sb[:], pv[:])
                nc.scalar.activation(vd_sb[:], v_sb[:], ACT.Copy,
                                     scale=1.0 / OS)

            stages = [(d8_sb if O8 else None, hb_cur, sc)]
            if prev is not None:
                stages = [prev]
                prev = (d8_sb if O8 else None, hb_cur, sc)
            elif nsc > 1:
                prev = (d8_sb if O8 else None, hb_cur, sc)
                stages = []
            for (p_d8, p_hb, p_sc) in stages:
                _emit_out(nc, p_d8, p_hb, p_sc, wo8_sb if O8 else None,
                          wob_sb, v_sb if O8 else None,
                          vd_sb if O8 else None, opool, spool, outT)
            h_prev = h_cur

          if prev is not None:
            (p_d8, p_hb, p_sc) = prev
            _emit_out(nc, p_d8, p_hb, p_sc, wo8_sb if O8 else None,
                      wob_sb, v_sb if O8 else None, vd_sb if O8 else None,
                      opool, spool, outT)

          if timing and _rep == reps - 1:
            tok = spool.tile([1, 8 * reps], F32, tag="tok")
            nc.vector.memset(tok[:], 1.0)
            nc.sync.dma_start(done[:], tok[:])

    nc.compile()
    return nc


def get_nc(seq_len=S, reps=1, timing=False):
    key = (seq_len, reps, timing)
    if key not in _cache:
        _cache[key] = _build(seq_len, reps, timing)
    return _cache[key]


def make_in_maps(x, W_hidden, W_gate, W_out):
    """Shard full inputs into per-core input maps (core c -> batch c//2,
    Di-half c%2)."""
    in_maps = []
    xT = np.ascontiguousarray(np.transpose(x, (0, 2, 1)))        # [B, D, S]
    xT8 = xT.astype(NP_FP8)
    xTb = xT.astype(NP_BF16)
    for c in range(N_CORES):
        b, hf = divmod(c, 2)
        Wg = W_gate[:, hf * F:(hf + 1) * F]
        Wh = W_hidden[:, hf * F:(hf + 1) * F]
        Wo = W_out[hf * F:(hf + 1) * F, :]
        m = {}
        if G8 or H8:
            m["x8"] = xT8[b]
        if G8 < KD or H8 < KD:
            m["xb"] = xTb[b]
        if G8:
            m["wg8"] = np.ascontiguousarray(Wg[:G8 * 128] * SW).astype(NP_FP8)
        if G8 < KD:
            m["wgb"] = np.ascontiguousarray(Wg[G8 * 128:]).astype(NP_BF16)
        if H8:
            m["wh8"] = np.ascontiguousarray(Wh[:H8 * 128] * SW).astype(NP_FP8)
        if H8 < KD:
            m["whb"] = np.ascontiguousarray(Wh[H8 * 128:]).astype(NP_BF16)
        if O8:
            m["wo8"] = np.ascontiguousarray(Wo[:O8 * 128] * SW).astype(NP_FP8)
        m["wob"] = np.ascontiguousarray(Wo * OS).astype(NP_BF16)
        in_maps.append(m)
    return in_maps


def assemble(results):
    """Combine per-core partial transposed outputs into [B, S, D]."""
    out = np.empty((B, S, D), np.float32)
    for b in range(B):
        acc = results[2 * b]["outT"] + results[2 * b + 1]["outT"]  # [D, S]
        out[b] = acc.T
    return out


def kernel(x, W_hidden, W_gate, W_out):
    x = np.asarray(x, np.float32)
    W_hidden = np.asarray(W_hidden, np.float32)
    W_gate = np.asarray(W_gate, np.float32)
    W_out = np.asarray(W_out, np.float32)
    nc = get_nc()
    in_maps = make_in_maps(x, W_hidden, W_gate, W_out)
    last_err = None
    for attempt in range(3):
        try:
            res = run_bass_kernel_spmd(nc, in_maps, list(range(N_CORES)))
            return assemble(res.results)
        except Exception as e:  # transient device faults under axon
            last_err = e
            import time as _time
            _time.sleep(5.0 * (attempt + 1))
    raise last_err
